# revision 18
# baseline (speedup 1.0000x reference)
"""KAN group-spline kernel for Trainium2 (8 NeuronCores, data-parallel over batch).

Math: out = id_gain[c]*x + F_c(v) + bias[c], v = 15.5*a*x + oc_c, F_c = channel's
cubic spline (32-knot uniform B-spline, constant outside v in [0,33]).

Exact device basis — clamped telescoped cubic pieces (baseline math, but each
knot fully fused into ONE DVE op and accumulated on the PE, not GPSIMD):

    v   = ig-affine input:  v = 15.5*a*x + oc_c                 [ACT, 1 pass]
    F_c = A_c + sum_S g_S(r_S),  r_S = clamp(v - S, 0, 1)
    g_S(r) = ((c3*r + c2)*r + c1)*r        per (channel, piece), host-computed

Saturation is automatic: v<=0 -> all r=0 -> A_c;  v>=33 -> all r=1 -> F(33).
Piece increments g_S are O(alpha) small -> safe in reduced-precision matmul.

Engine plan per 128xNCOL tile (knot ops independent -> full pipelining):
  ACT : v = Identity(scale*x + oc)                  [per-partition bias AP]
  PE  : psum  = diag(ig) @ x_chunk   [fp32, start]  \  init acc0
        psum += diag(bias+A) @ ones  [fp32]         /
  DVE : y_S = g_S(clamp(v - S, 0, 1))   [custom 8-stage TTSS op, 1-tensor ->
        full rate; s0=c1 AP, s1=c2 AP, in1=c3 spill AP, imm2=S]
  PE  : psum += I @ y_S_chunk     [float32r -> 1 cyc/col, full rate]
  ACT : out_sbuf = Copy(psum)                       [evacuate]
DVE never streams two SBUF tensors (S2S2D2_STT 2-source = half rate) and the
PE never runs plain-fp32 streams on the hot path (fp32 matmul = 1/4 rate).
"""

import os
import numpy as np

B, C, H, W = 16, 192, 128, 128
K, G = 32, 32
NCORES = 8
NSEG = 33
ROWS = (B // NCORES) * C           # 384 rows per core
FREE = H * W                       # 16384
NCOL = int(os.environ.get("KAN_NCOL", "4096"))
COLT = FREE // NCOL
ROWT = ROWS // 128                 # 3
MMF = 512                          # matmul free chunk / PSUM bank (fp32 out)
NMM = NCOL // MMF

OFF_OC, OFF_C1, OFF_C2, OFF_C3 = 0, 1, 1 + NSEG, 1 + 2 * NSEG
NTAB = 1 + 3 * NSEG
NWTS = 2 * ROWT                    # per-rowtile {ig, bias2} diags (fp32)

_BMAT = np.array(
    [
        [1 / 6, -3 / 6, 3 / 6, -1 / 6],
        [4 / 6, 0.0, -6 / 6, 3 / 6],
        [1 / 6, 3 / 6, 3 / 6, -3 / 6],
        [0.0, 0.0, 0.0, 1 / 6],
    ],
    dtype=np.float64,
)  # [tap k, power m]


def build_tables(alpha, a, b, id_gain, bias, group_idx):
    """Per-(channel,piece) telescoped cubic coefficients + affine params.
    Returns (scale, tab[ROWT,128,NTAB], wts[NWTS,128,128])."""
    g = group_idx.astype(np.int64)
    alpha_pc = alpha.astype(np.float64)[g]                      # (C, K)
    a64, b64 = a.astype(np.float64), b.astype(np.float64)
    assert np.all(a64 == a64[0]), "fast path needs uniform a (ACT scale is imm)"
    scale = 15.5 * a64[0]
    oc = 15.5 * (b64 + 1.0) + 1.0                               # (C,)

    S = np.arange(NSEG)
    taps = np.clip(S[:, None] - 2 + np.arange(4)[None, :], 0, K - 1)
    A = alpha_pc[:, taps]                                       # (C, NSEG, 4)
    P = np.einsum("csk,km->csm", A, _BMAT)                      # (C, NSEG, 4)
    c1, c2, c3 = P[..., 1], P[..., 2], P[..., 3]
    Ac = P[:, 0, 0]
    bias2 = bias.astype(np.float64) + Ac

    tab = np.zeros((ROWT, 128, NTAB), dtype=np.float64)
    wts = np.zeros((NWTS, 128, 128), dtype=np.float64)
    for t in range(ROWT):
        ch = (t * 128 + np.arange(128)) % C
        tab[t, :, OFF_OC] = oc[ch]
        tab[t, :, OFF_C1:OFF_C1 + NSEG] = c1[ch]
        tab[t, :, OFF_C2:OFF_C2 + NSEG] = c2[ch]
        tab[t, :, OFF_C3:OFF_C3 + NSEG] = c3[ch]
        wts[2 * t] = np.diag(id_gain.astype(np.float64)[ch])
        wts[2 * t + 1] = np.diag(bias2[ch])
    return np.float32(scale), tab.astype(np.float32), wts.astype(np.float32)


def host_emulate(x_rows, scale, tab_t, ig_diag, b2_diag):
    """Numpy fp32 emulation of the device program for one row-tile."""
    f = np.float32
    v = f(scale) * x_rows + tab_t[:, OFF_OC, None]
    acc = np.diag(ig_diag)[:, None] * x_rows + np.diag(b2_diag)[:, None]
    for s in range(NSEG):
        r = np.clip(v - f(s), f(0), f(1))
        c1 = tab_t[:, OFF_C1 + s, None]
        c2 = tab_t[:, OFF_C2 + s, None]
        c3 = tab_t[:, OFF_C3 + s, None]
        acc = acc + ((c3 * r + c2) * r + c1) * r
    return acc


_PROG_CACHE = {}


def _get_custom_op():
    from concourse.dve_spec import (Spec, Src0, C0, C1, C2, C3, One, relu,
                                    minn, lower, _spill_c3_to_src1)
    from concourse import dve_ops
    from concourse.dve_ops import DveOp, OPS
    from concourse.dve_uop import DveOpSpec

    for op in OPS:
        if op.name == "KAN_TEL":
            return op

    r = minn(relu(Src0 - C2), One)
    body = _spill_c3_to_src1(((C3 * r + C1) * r + C0) * r)

    def ref(in0, in1, s0, s1, imm2):
        rr = np.clip(in0 - np.float32(imm2), np.float32(0), np.float32(1)).astype(np.float32)
        return ((in1 * rr + s1) * rr + s0) * rr

    spec = Spec(body=body, reference=ref)
    shas = {}
    for ver in ("v3", "v4"):
        tmp = DveOpSpec(name="KAN_TEL", opcode=0, uops=lower(spec, ver=ver), rd1_en=True)
        shas[ver] = tmp.sha(ver)
    op = DveOp("KAN_TEL", spec, subdim=False, uops_sha=shas)
    row = dve_ops._CUSTOM_DVE_ROW_BASE + len(OPS)
    assert row < 0x20
    OPS.append(op)
    dve_ops.CUSTOM_DVE_SPECS[op.name] = spec
    dve_ops._SUB_OPCODE_FOR_NAME[op.name] = row
    assert dve_ops.get_dve_sub_opcode("KAN_TEL") == row
    return op


def _build_program(scale):
    repeat = int(os.environ.get("KAN_REPEAT", "1"))
    key = ("prog", float(scale), NCOL, repeat, os.environ.get("KAN_F32R", "1"))
    if key in _PROG_CACHE:
        return _PROG_CACHE[key]

    import concourse.bacc as bacc
    import concourse.mybir as mybir
    from concourse.tile import TileContext

    kan_op = _get_custom_op()

    nc = bacc.Bacc("TRN2", target_bir_lowering=False, debug=False, num_devices=NCORES)
    x_d = nc.dram_tensor("x", [ROWS, FREE], mybir.dt.float32, kind="ExternalInput").ap()
    tab_d = nc.dram_tensor("tab", [ROWT * 128, NTAB], mybir.dt.float32, kind="ExternalInput").ap()
    wts_d = nc.dram_tensor("wts", [NWTS * 128, 128], mybir.dt.float32, kind="ExternalInput").ap()
    idb_d = nc.dram_tensor("identb", [128, 128], mybir.dt.bfloat16, kind="ExternalInput").ap()
    out_d = nc.dram_tensor("out", [ROWS, FREE], mybir.dt.float32, kind="ExternalOutput").ap()

    with TileContext(nc) as tc:
        with (
            tc.tile_pool(name="tabp", bufs=ROWT) as tabp,
            tc.tile_pool(name="wtsp", bufs=NWTS + 1) as wtsp,
            tc.tile_pool(name="onesp", bufs=1) as onesp,
            tc.tile_pool(name="xp", bufs=2) as xp,
            tc.tile_pool(name="vp", bufs=2) as vp,
            tc.tile_pool(name="yp", bufs=int(os.environ.get("KAN_YBUFS", "4"))) as yp,
            tc.tile_pool(name="outp", bufs=2) as outp,
            tc.tile_pool(name="psp", bufs=1, space="PSUM") as psp,
        ):
            tabs, wtss = [], []
            for t in range(ROWT):
                tt = tabp.tile([128, NTAB], mybir.dt.float32, tag="tab")
                nc.sync.dma_start(tt[:], tab_d[t * 128:(t + 1) * 128, :])
                tabs.append(tt)
            for i in range(NWTS):
                wt_ = wtsp.tile([128, 128], mybir.dt.float32, tag="wts")
                nc.sync.dma_start(wt_[:], wts_d[i * 128:(i + 1) * 128, :])
                wtss.append(wt_)
            identb = wtsp.tile([128, 128], mybir.dt.bfloat16, tag="identb")
            nc.sync.dma_start(identb[:], idb_d[:, :])
            wtss.append(identb)
            ones = onesp.tile([128, MMF], mybir.dt.float32, tag="ones")
            nc.vector.memset(ones[:], 1.0)

            import contextlib
            loop_ctx = tc.For_i(0, repeat, 1) if repeat > 1 else contextlib.nullcontext()
            with loop_ctx:
                _emit_body(nc, tc, tabs, wtss, ones, x_d, out_d, xp, vp, yp, outp, psp, kan_op, scale)

    nc.compile()
    _PROG_CACHE[key] = nc
    return nc


def _emit_body(nc, tc, tabs, wtss, ones, x_d, out_d, xp, vp, yp, outp, psp, kan_op, scale):
    import concourse.mybir as mybir

    ident_f = mybir.ActivationFunctionType.Identity
    copy_f = mybir.ActivationFunctionType.Copy
    YDT = mybir.dt.bfloat16
    ident = wtss[-1]
    for t in range(ROWT):
        tt = tabs[t]
        igd, b2d = wtss[2 * t], wtss[2 * t + 1]
        for j in range(COLT):
            rs, cs = slice(t * 128, (t + 1) * 128), slice(j * NCOL, (j + 1) * NCOL)
            xt = xp.tile([128, NCOL], mybir.dt.float32, tag="x")
            nc.sync.dma_start(xt[:], x_d[rs, cs])
            vt = vp.tile([128, NCOL], mybir.dt.float32, tag="v")
            nc.scalar.activation(
                vt[:], xt[:], ident_f,
                bias=tt[:, OFF_OC:OFF_OC + 1], scale=float(scale),
            )
            ps = psp.tile([128, NCOL], mybir.dt.float32, tag="ps")
            for m in range(NMM):
                ms = slice(m * MMF, (m + 1) * MMF)
                nc.tensor.matmul(ps[:, ms], igd[:], xt[:, ms], start=True, stop=False)
                nc.tensor.matmul(ps[:, ms], b2d[:], ones[:], start=False, stop=False)
            for s in range(NSEG):
                y = yp.tile([128, NCOL], YDT, tag="y")
                nc.vector._custom_dve(
                    kan_op, out=y[:], in0=vt[:],
                    in1=tt[:, OFF_C3 + s:OFF_C3 + s + 1],
                    s0=tt[:, OFF_C1 + s:OFF_C1 + s + 1],
                    s1=tt[:, OFF_C2 + s:OFF_C2 + s + 1],
                    imm2=float(s),
                )
                last = s == NSEG - 1
                for m in range(NMM):
                    ms = slice(m * MMF, (m + 1) * MMF)
                    nc.tensor.matmul(
                        ps[:, ms], ident[:], y[:, ms], start=False, stop=last,
                    )
            outt = outp.tile([128, NCOL], mybir.dt.float32, tag="out")
            nc.scalar.activation(outt[:], ps[:], copy_f, bias=0.0)
            nc.sync.dma_start(out_d[rs, cs], outt[:])


def kernel(**inputs):
    x = np.asarray(inputs["x"], dtype=np.float32)
    scale, tab, wts = build_tables(
        np.asarray(inputs["alpha"]), np.asarray(inputs["a"]), np.asarray(inputs["b"]),
        np.asarray(inputs["id_gain"]), np.asarray(inputs["bias"]),
        np.asarray(inputs["group_idx"]),
    )
    from concourse import bass_utils

    nc = _build_program(scale)
    tab_flat = np.ascontiguousarray(tab.reshape(ROWT * 128, NTAB))
    wts_flat = np.ascontiguousarray(wts.reshape(NWTS * 128, 128))
    xs = x.reshape(NCORES, B // NCORES, C, H, W)
    import ml_dtypes
    identb = np.eye(128, dtype=ml_dtypes.bfloat16)
    in_maps = [
        {"x": np.ascontiguousarray(xs[i].reshape(ROWS, FREE)), "tab": tab_flat,
         "wts": wts_flat, "identb": identb}
        for i in range(NCORES)
    ]
    trace = bool(int(os.environ.get("KAN_TRACE", "0")))
    res = bass_utils.run_bass_kernel_spmd(
        nc, in_maps, list(range(NCORES)), trace=trace,
        tmpdir=os.environ.get("KAN_TMPDIR") or None,
    )
    if trace and res.exec_time_ns is not None:
        print(f"HW exec time: {res.exec_time_ns} ns")
    out = np.stack([res.results[i]["out"] for i in range(NCORES)])
    return np.ascontiguousarray(out.reshape(B, C, H, W).astype(np.float32))
